# revision 33
# baseline (speedup 1.0000x reference)
"""Dense correspondence contrastive loss kernel for Trainium2 (8 NeuronCores).

Problem (B=32, C=64, N=1024 spatial positions per sample):
  - l2-normalize q_b/k_b/q_grid/k_grid along C
  - sim[b,i,j] = <qb_hat[b,:,i], kb_hat[b,:,j]>; idx = argmax_j sim
  - pos[b,i] = <qg_hat[b,:,i], kg_hat[b,:,idx[b,i]]> / 0.1
  - neg[b,i] = <qg_hat[b,:,i], kg_hat[neg_idx[b],:,i]> / 0.1
    (neg_idx from labels/neg_noise -- O(B^2) host-side index prep)
  - loss = mean(log(exp(pos)+exp(neg)+1e-6) - pos)

Sharding: data-parallel over batch, 4 samples per core.

v2 design (per core):
  - q_b normalization skipped (argmax invariant to per-row scale); k_b
    column norms rn[j] = 1/|k_b[:,j]| computed via PE colsum of squares
    into a partition-parallel [8,128] layout, then reshaped/broadcast to
    a [128,1024] SBUF tile with two small DMAs.
  - sim matmuls run on RAW fp32 operands in float32r mode (1 cyc/row at
    >=256 moving); no bf16 conversion passes.
  - single-pass fused argmax: a custom DVE op computes
    argmax_j(sim[i,j]*rn[j]) in ONE pass over PSUM (running scan-max +
    index select + MAX-accumulator), replacing the reduce_max + STT pair.
  - ONE batched indirect DMA per sample gathers all 8 m-chunks of
    matched k_grid rows (1024 descriptors, single SWDGE fixed cost).
  - grid tail: ACT squares + Pool products into a [128,2560] scratch,
    strided DVE reduces into kind-major accumulators, batched loss tail.
"""

import os
import numpy as np

B = 32
C = 64
N = 1024
NCORES = 8
SPC = B // NCORES          # samples per core
MT = N // 128              # 128-row m-tiles per sample
NT = SPC * MT              # accumulator columns per core
TEMP = 0.1
EPS_LOSS = 1e-6

USE_F32R = False           # fp32r sim matmuls rejected by BIR verifier
                           # ("not rounded to FP32r"); bf16 copies instead

LAST_EXEC_TIME_NS = None
_CACHE = {}


def _ensure_ntff_hook():
    """Some agent images ship only the antenv stub (no axon_hooks); bass_utils
    then crashes on `from antenv.axon_hooks import ...` when tracing under
    axon.  Install a functional shim wired to the libaxon ctypes hook so NTFF
    profiling (and exec_time_ns) works.  No-op when the real module exists."""
    import sys
    import types
    try:
        import antenv.axon_hooks  # noqa: F401
        return
    except ImportError:
        pass
    try:
        import antenv
    except ImportError:
        return
    mod = types.ModuleType("antenv.axon_hooks")
    mod._hook = None

    def set_axon_ntff_profile_hook(h):
        mod._hook = h

    def get_axon_ntff_profile_hook():
        return mod._hook

    mod.set_axon_ntff_profile_hook = set_axon_ntff_profile_hook
    mod.get_axon_ntff_profile_hook = get_axon_ntff_profile_hook
    sys.modules["antenv.axon_hooks"] = mod
    antenv.axon_hooks = mod
    try:
        from trn_agent_boot.trn_boot import _ntff_profile_via_ctypes
        hook = _ntff_profile_via_ctypes("/opt/axon/libaxon_pjrt.so")
        if hook is not None:
            mod._hook = hook
    except Exception:
        pass


def _register_argmax_op():
    """Register a custom DVE op: single-pass scaled argmax.

    out[k]    = select(v_k >= runmax(v)_k, k, -FLT_MAX),  v = in0*in1
    accum_out = max_k out[k]   (== argmax_k v; last index on exact ties,
                                but fp32 exact ties have ~0 probability)
    """
    from concourse import dve_ops
    from concourse.dve_spec import (
        Spec, lower, Src0, Src1, scan, Idx, select, AluOp, MaxNeg, _has_src1,
    )
    from concourse.dve_uop import DveOpSpec
    from concourse.dve_ops import DveOp

    name = "ARGMAX_SCALED_ANT"
    for op in dve_ops.OPS:
        if op.name == name:
            return op

    def ref(in0, in1, c0, c1, c2):
        p = in0.shape[0]
        a = np.asarray(in0, np.float32).reshape(p, -1)
        bmat = np.asarray(in1, np.float32).reshape(p, -1)
        v = a * bmat
        run = np.maximum.accumulate(v, axis=1)
        cond = v >= run
        idxs = np.arange(a.shape[1], dtype=np.float32)[None, :]
        out = np.where(cond, idxs, np.float32(-3.4028234663852886e38))
        acc = out.max(axis=1)
        return out.reshape(in0.shape), acc

    v = Src0 * Src1
    body = select(v >= scan(AluOp.MAX, v), Idx, MaxNeg)
    spec = Spec(body=body, accum=AluOp.MAX, reference=ref)

    row = max(dve_ops._SUB_OPCODE_FOR_NAME.values()) + 1
    assert row < 0x20
    dve_ops._SUB_OPCODE_FOR_NAME[name] = row
    shas = {}
    for ver in ("v3", "v4"):
        try:
            tmp = DveOpSpec(name=name, opcode=row, uops=lower(spec, ver=ver),
                            rd1_en=_has_src1(spec))
            shas[ver] = tmp.sha(ver)
        except Exception:
            pass
    op = DveOp(name, spec, subdim=False, uops_sha=shas)
    dve_ops.OPS.append(op)
    dve_ops.CUSTOM_DVE_SPECS[name] = spec
    return op


def _build_module():
    import concourse.bass as bass
    import concourse.bacc as bacc
    import concourse.tile as tile
    from concourse import mybir
    from contextlib import ExitStack

    argmax_op = _register_argmax_op()

    F32 = mybir.dt.float32
    F32R = mybir.dt.float32r
    BF16 = mybir.dt.bfloat16
    FP16 = mybir.dt.float16
    U32 = mybir.dt.uint32
    AX = mybir.AxisListType
    ALU = mybir.AluOpType
    ACTF = mybir.ActivationFunctionType

    nc = bacc.Bacc("TRN2", target_bir_lowering=False, debug=False,
                   num_devices=NCORES)

    qb_d = nc.dram_tensor("qb", [SPC * C, N], F32, kind="ExternalInput")
    kb_d = nc.dram_tensor("kb", [SPC * C, N], F32, kind="ExternalInput")
    qgt_d = nc.dram_tensor("qgt", [128, SPC * MT * C], F32, kind="ExternalInput")
    kgt_d = nc.dram_tensor("kgt", [SPC * N, C], F32, kind="ExternalInput")
    kngt_d = nc.dram_tensor("kngt", [128, SPC * MT * C], F32, kind="ExternalInput")
    ind_d = nc.dram_tensor("cst_ind", [C, MT * MT], BF16, kind="ExternalInput")
    rnd_d = nc.dram_tensor("rnd", [SPC, N], F32, kind="Internal")
    out_d = nc.dram_tensor("out", [1, 1], F32, kind="ExternalOutput")

    with tile.TileContext(nc) as tc, ExitStack() as ctx:
        const = ctx.enter_context(tc.tile_pool(name="const", bufs=1))
        accum = ctx.enter_context(tc.tile_pool(name="accum", bufs=1))
        io = ctx.enter_context(tc.tile_pool(name="io", bufs=3))
        qg_p = ctx.enter_context(tc.tile_pool(name="qg", bufs=4))
        mt_p = ctx.enter_context(tc.tile_pool(name="mt", bufs=4))
        scr = ctx.enter_context(tc.tile_pool(name="scr", bufs=2))
        ps_sim = ctx.enter_context(tc.tile_pool(name="ps_sim", bufs=3, space="PSUM"))
        ps_aux = ctx.enter_context(tc.tile_pool(name="ps_aux", bufs=2, space="PSUM"))

        ones128 = const.tile([128, 1], F32)
        nc.vector.memset(ones128[:], 1.0)
        b24 = const.tile([MT, 1], F32)
        nc.vector.memset(b24[:], 1e-24)
        b24t = const.tile([128, 1], F32)
        nc.vector.memset(b24t[:], 1e-24)
        # dummy sqrt pins the initial ACT table to the sqrt set (which also
        # holds Square and Copy) -- avoids a 1.3us table swap on the first
        # sample's norm critical path
        dum = const.tile([1, 1], F32)
        nc.scalar.activation(dum[:], b24t[0:1, :], ACTF.Sqrt)
        # chunk-indicator weights: colsum of sq chunk j lands in PSUM
        # partition j (PE output base partition must be 0)
        ind_sb = const.tile([C, MT * MT], BF16)
        nc.sync.dma_start(ind_sb[:], ind_d[:, :])
        inds = [ind_sb[:, j * MT:(j + 1) * MT] for j in range(MT)]

        # kind-major accumulator for the batched loss tail; blocks of NT cols:
        # [dns | qg^2 | kng^2 | kga^2 | dps] so the early reduce covers blocks
        # 0:3 and the late reduce blocks 3:5, while sqrt/recip see the three
        # ssq blocks 1:4 as one contiguous [128, 3*NT] range
        acc = accum.tile([128, 5 * NT], F32, tag="acc")
        accv = acc[:].rearrange("p (k x) -> p k x", k=5)

        import concourse.bass as bass_mod

        def emit_load(b):
            # kb halves on both HWDGE rings (kb heads the norm critical path)
            st = {}
            kb_t = io.tile([C, N], F32, tag="kb")
            nc.sync.dma_start(kb_t[:, 0:512], kb_d[b * C:(b + 1) * C, 0:512])
            nc.scalar.dma_start(kb_t[:, 512:N], kb_d[b * C:(b + 1) * C, 512:N])
            qb_t = io.tile([C, N], F32, tag="qb")
            nc.scalar.dma_start(qb_t[:], qb_d[b * C:(b + 1) * C, :])
            qgs = qg_p.tile([128, MT * C], F32, tag="qg")
            nc.sync.dma_start(qgs[:], qgt_d[:, b * MT * C:(b + 1) * MT * C])
            kngs = qg_p.tile([128, MT * C], F32, tag="kng")
            nc.sync.dma_start(kngs[:], kngt_d[:, b * MT * C:(b + 1) * MT * C])
            st["kb_t"], st["qb_t"], st["qgs"], st["kngs"] = kb_t, qb_t, qgs, kngs
            return st

        def emit_norm_a(b, st):
            # squares + bf16 copies first: ACT is in-order, and the copies
            # gate the next sample's sim matmuls on PE
            sq = io.tile([C, N], BF16, tag="sq")
            nc.scalar.activation(sq[:, 0:512], st["kb_t"][:, 0:512], ACTF.Square)
            nc.scalar.activation(sq[:, 512:N], st["kb_t"][:, 512:N], ACTF.Square)
            st["sq"] = sq
            if not USE_F32R:
                kb_bf = io.tile([C, N], BF16, tag="kb_bf")
                nc.scalar.activation(kb_bf[:], st["kb_t"][:], ACTF.Copy)
                qb_bf = io.tile([C, N], BF16, tag="qb_bf")
                nc.scalar.activation(qb_bf[:], st["qb_t"][:], ACTF.Copy)
                st["qb_bf"], st["kb_bf"] = qb_bf, kb_bf
            return st

        def emit_norm_b(b, st):
            # k_b column norms -> reciprocal -> [128, N] broadcast tile for
            # the fused argmax multiplier (broadcast via a DRAM bounce: write
            # the [8,128] recip row-major to DRAM, read it back 128x)
            sq = st.pop("sq")
            ssq_ps = ps_aux.tile([MT, 128], F32, tag="aux")
            for j in range(MT):
                nc.tensor.matmul(ssq_ps[:], inds[j],
                                 sq[:, j * 128:(j + 1) * 128],
                                 start=(j == 0), stop=(j == MT - 1))
            rn_s = io.tile([MT, 128], F32, tag="rn_s")
            nc.scalar.activation(rn_s[:], ssq_ps[:], ACTF.Sqrt, bias=b24[:])
            rn8 = io.tile([MT, 128], F32, tag="rn8")
            nc.vector.reciprocal(rn8[:], rn_s[:])
            nc.sync.dma_start(rnd_d[b:b + 1, :], rn8[:, :])
            rnb_sb = io.tile([128, N], F32, tag="rnb_sb")
            nc.sync.dma_start(rnb_sb[:], rnd_d[b:b + 1, :].broadcast_to([128, N]))
            st["rnb"] = rnb_sb
            st["idxf"] = mt_p.tile([128, MT], F32, tag="idxf", name=f"idxf{b}")
            return st

        def emit_mtile(b, m, st):
            sim_ps = ps_sim.tile([128, N], F32, tag="sim")
            if USE_F32R:
                lhs = st["qb_t"][:, m * 128:(m + 1) * 128].bitcast(F32R)
                rhs0 = st["kb_t"][:, 0:512].bitcast(F32R)
                rhs1 = st["kb_t"][:, 512:N].bitcast(F32R)
            else:
                lhs = st["qb_bf"][:, m * 128:(m + 1) * 128]
                rhs0 = st["kb_bf"][:, 0:512]
                rhs1 = st["kb_bf"][:, 512:N]
            nc.tensor.matmul(sim_ps[:, 0:512], lhs, rhs0, start=True, stop=True)
            nc.tensor.matmul(sim_ps[:, 512:N], lhs, rhs1, start=True, stop=True)
            scrap = scr.tile([128, N], FP16, tag="scrap")
            nc.vector._custom_dve(
                argmax_op, out=scrap[:], in0=sim_ps[:], in1=st["rnb"][:],
                accum_out=st["idxf"][:, m:m + 1])

        def emit_gather(b, st, mlo, mhi):
            # index prep rides the Pool engine (DVE is the bottleneck)
            idxc = mt_p.tile([128, mhi - mlo], F32, tag="idxc")
            nc.gpsimd.tensor_scalar(idxc[:], st["idxf"][:, mlo:mhi],
                                    float(b * N), 0.0, op0=ALU.add, op1=ALU.add)
            idxu = mt_p.tile([128, mhi - mlo], U32, tag="idxu")
            nc.gpsimd.tensor_copy(idxu[:], idxc[:])
            nc.gpsimd.indirect_dma_start(
                st["kgas"][:, mlo * C:mhi * C], None, kgt_d.ap(),
                bass_mod.IndirectOffsetOnAxis(ap=idxu[:, 0:mhi - mlo], axis=0))

        # big5 scratch layout: [prodn | qg^2 | kng^2 | kga^2 | prodp], matching
        # the acc block order so one strided reduce covers each phase
        def emit_prods_a(b, st):
            # gather-independent products/squares (ACT + Pool only)
            big5 = scr.tile([128, 5 * MT * C], F32, tag="big5", name=f"big5{b}")
            st["big5"] = big5
            nc.gpsimd.tensor_mul(big5[:, 0:512], st["qgs"][:], st["kngs"][:])
            nc.scalar.activation(big5[:, 512:1024], st["qgs"][:], ACTF.Square)
            nc.scalar.activation(big5[:, 1024:1536], st["kngs"][:], ACTF.Square)

        def emit_prods_b(b, st, mlo, mhi):
            # gather-dependent products/squares (ACT + Pool only)
            big5 = st["big5"]
            lo, hi = mlo * C, mhi * C
            nc.scalar.activation(big5[:, 1536 + lo:1536 + hi],
                                 st["kgas"][:, lo:hi], ACTF.Square)
            nc.gpsimd.tensor_mul(big5[:, 2048 + lo:2048 + hi],
                                 st["qgs"][:, lo:hi], st["kgas"][:, lo:hi])

        def emit_early_reduce(b, st):
            nc.vector.tensor_reduce(
                accv[:, 0:3, b * MT:(b + 1) * MT],
                st["big5"][:, 0:1536].rearrange("p (k c) -> p k c", c=C),
                axis=AX.X, op=ALU.add)

        def emit_late_reduce(b, st, mlo, mhi):
            nc.vector.tensor_reduce(
                accv[:, 3:5, b * MT + mlo:b * MT + mhi],
                st["big5"][:, 1536:2560].rearrange("p (k m c) -> p k m c", k=2, c=C)
                [:, :, mlo:mhi, :],
                axis=AX.X, op=ALU.add)

        # loss-tail tiles (filled in two chunks: samples 0..2, then sample 3,
        # so only the last sample's slice sits in the pipeline drain; exp/ln
        # run once at the end so only one act-table swap occurs)
        ra_s = accum.tile([128, 3 * NT], F32, tag="ra_s")
        ra = accum.tile([128, 3 * NT], F32, tag="ra")
        pos = accum.tile([128, NT], F32, tag="pos")
        ngv = accum.tile([128, NT], F32, tag="ngv")

        def emit_tail_chunk(lo, hi):
            """pos/neg logits for per-sample columns [lo, hi) of NT=32."""
            w = hi - lo
            rsv = ra_s[:].rearrange("p (k x) -> p k x", k=3)
            rav = ra[:].rearrange("p (k x) -> p k x", k=3)
            nc.scalar.activation(rsv[:, :, lo:hi], accv[:, 1:4, lo:hi],
                                 ACTF.Sqrt, bias=b24t[:])
            nc.vector.reciprocal(rav[:, :, lo:hi], rsv[:, :, lo:hi])
            rq = ra[:, lo:hi]
            rkng = ra[:, NT + lo:NT + hi]
            rkga = ra[:, 2 * NT + lo:2 * NT + hi]
            t1 = mt_p.tile([128, w], F32, tag="t1")
            nc.vector.tensor_mul(t1[:], accv[:, 4, lo:hi], rq)
            nc.vector.scalar_tensor_tensor(pos[:, lo:hi], t1[:], 10.0, rkga,
                                           op0=ALU.mult, op1=ALU.mult)
            t2 = mt_p.tile([128, w], F32, tag="t2")
            nc.vector.tensor_mul(t2[:], accv[:, 0, lo:hi], rq)
            nc.vector.scalar_tensor_tensor(ngv[:, lo:hi], t2[:], 10.0, rkng,
                                           op0=ALU.mult, op1=ALU.mult)

        # software-pipelined emission, 2-deep on the norm chain: sample b+2's
        # loads + norm run during sample b, so the argmax multiplier tile is
        # ready with several m-tiles of slack however the Tile scheduler
        # orders things.  ALL of the last sample's reduces + tail run in the
        # drain window, overlapped with its final gathers.
        st = emit_load(0)
        emit_norm_a(0, st)
        emit_norm_b(0, st)
        states = {0: st}
        if SPC > 1:
            st1 = emit_load(1)
            emit_norm_a(1, st1)
            emit_norm_b(1, st1)
            states[1] = st1
        for b in range(SPC):
            cur = states.pop(b)
            cur["kgas"] = qg_p.tile([128, MT * C], F32, tag="kga", name=f"kgas{b}")
            last = b == SPC - 1
            for m in range(MT):
                emit_mtile(b, m, cur)
                if b + 2 < SPC:
                    if m == 0:
                        states[b + 2] = emit_load(b + 2)
                    if m == 2:
                        emit_norm_a(b + 2, states[b + 2])
                    if m == 4:
                        emit_norm_b(b + 2, states[b + 2])
                if m == 1 and last:
                    emit_gather(b, cur, 0, 2)
                if m == 2 and b > 0:
                    prev = states[("done", b - 1)]
                    emit_prods_b(b - 1, prev, 0, MT)
                if m == 3:
                    emit_prods_a(b, cur)
                    if last:
                        emit_gather(b, cur, 2, 4)
                        emit_prods_b(b, cur, 0, 2)
                if m == 5 and last:
                    emit_gather(b, cur, 4, 6)
                    emit_prods_b(b, cur, 2, 4)
                if m == 6 and b > 0 and not last:
                    prev = states.pop(("done", b - 1))
                    emit_early_reduce(b - 1, prev)
                    emit_late_reduce(b - 1, prev, 0, MT)
                if m == 7 and last:
                    emit_gather(b, cur, 6, 7)
            if last:
                # drain: the final 128-row gather chain (Pool/DMA/ACT) runs
                # in parallel with the accumulated DVE reduce + tail work
                emit_gather(b, cur, 7, MT)
                emit_prods_b(b, cur, 4, 6)
                prev = states.pop(("done", b - 1))
                emit_early_reduce(b - 1, prev)
                emit_late_reduce(b - 1, prev, 0, MT)
                emit_early_reduce(b, cur)
                emit_late_reduce(b, cur, 0, 4)
                emit_prods_b(b, cur, 6, 7)
                emit_late_reduce(b, cur, 4, 6)
                emit_tail_chunk(0, 3 * MT)
                emit_prods_b(b, cur, 7, MT)
                emit_late_reduce(b, cur, 6, MT)
                emit_tail_chunk(3 * MT, NT)
            else:
                emit_gather(b, cur, 0, MT)
                states[("done", b)] = cur

        ep = accum.tile([128, NT], F32, tag="ep")
        nc.scalar.activation(ep[:], pos[:], ACTF.Exp)
        en = accum.tile([128, NT], F32, tag="en")
        nc.scalar.activation(en[:], ngv[:], ACTF.Exp)
        ssum = accum.tile([128, NT], F32, tag="ssum")
        nc.vector.scalar_tensor_tensor(ssum[:], ep[:], EPS_LOSS, en[:],
                                       op0=ALU.add, op1=ALU.add)
        lg = accum.tile([128, NT], F32, tag="lg")
        nc.scalar.activation(lg[:], ssum[:], ACTF.Ln)
        li = accum.tile([128, NT], F32, tag="li")
        nc.vector.tensor_sub(li[:], lg[:], pos[:])
        lsum = accum.tile([128, 1], F32, tag="lsum")
        nc.vector.reduce_sum(lsum[:], li[:], axis=AX.X)

        tot_ps = ps_aux.tile([1, 1], F32, tag="aux")
        nc.tensor.matmul(tot_ps[:], lsum[:], ones128[:], start=True, stop=True)
        outt = mt_p.tile([1, 1], F32, tag="outt")
        nc.scalar.activation(outt[:], tot_ps[:], ACTF.Copy)
        nc.sync.dma_start(out_d[:, :], outt[:])

    nc.compile()
    return nc


def get_module():
    if "nc" not in _CACHE:
        _CACHE["nc"] = _build_module()
    return _CACHE["nc"]


def make_in_maps(q_b, k_b, q_grid, k_grid, labels, neg_noise):
    import ml_dtypes

    q_b = np.ascontiguousarray(np.asarray(q_b, dtype=np.float32)).reshape(B, C, N)
    k_b = np.ascontiguousarray(np.asarray(k_b, dtype=np.float32)).reshape(B, C, N)
    q_grid = np.ascontiguousarray(np.asarray(q_grid, dtype=np.float32)).reshape(B, C, N)
    k_grid = np.ascontiguousarray(np.asarray(k_grid, dtype=np.float32)).reshape(B, C, N)
    labels = np.asarray(labels)
    neg_noise = np.asarray(neg_noise, dtype=np.float32)

    # negative-sample index prep (O(B^2), matches jnp argmax tie-breaking)
    mask = labels[None, :] != labels[:, None]
    scores = np.where(mask, neg_noise, -np.inf)
    neg_idx = np.argmax(scores, axis=1)
    kng = k_grid[neg_idx]  # [B, C, N]

    mt = N // 128
    cst_ind = np.zeros((C, mt, mt), dtype=np.float32)
    for j in range(mt):
        cst_ind[:, j, j] = 1.0
    cst_ind = cst_ind.reshape(C, mt * mt).astype(ml_dtypes.bfloat16)

    def grid_rows(x):
        # [SPC, C, N] -> [128, SPC*MT*C]: partition p holds rows (b, m, c)
        # = x[b, c, m*128 + p]; one contiguous [128, 512] DMA per sample
        mt = N // 128
        return np.ascontiguousarray(
            x.reshape(SPC, C, mt, 128).transpose(3, 0, 2, 1)
        ).reshape(128, SPC * mt * C)

    in_maps = []
    for ci in range(NCORES):
        sl = slice(ci * SPC, (ci + 1) * SPC)
        in_maps.append({
            "qb": np.ascontiguousarray(q_b[sl]).reshape(SPC * C, N),
            "kb": np.ascontiguousarray(k_b[sl]).reshape(SPC * C, N),
            "qgt": grid_rows(q_grid[sl]),
            "kgt": np.ascontiguousarray(k_grid[sl].transpose(0, 2, 1)).reshape(SPC * N, C),
            "kngt": grid_rows(kng[sl]),
            "cst_ind": cst_ind,
        })
    return in_maps


def kernel(q_b, k_b, q_grid, k_grid, labels, neg_noise):
    global LAST_EXEC_TIME_NS
    _ensure_ntff_hook()
    in_maps = make_in_maps(q_b, k_b, q_grid, k_grid, labels, neg_noise)
    nc = get_module()
    from concourse.bass_utils import run_bass_kernel_spmd
    res = run_bass_kernel_spmd(nc, in_maps, core_ids=list(range(NCORES)))
    LAST_EXEC_TIME_NS = res.exec_time_ns
    total = sum(float(res.results[i]["out"][0, 0]) for i in range(NCORES))
    return np.float32(total / float(B * N))
